# revision 15
# baseline (speedup 1.0000x reference)
"""AttentionBlock (GroupNorm + MHSA + proj + residual) on 8 TRN2 NeuronCores.

Sharding: data-parallel over batch (B=8 -> 1 batch element per core), SPMD —
one Bass program, per-core input maps.

Per-core math (C=512, T=1024, 8 heads, ch=64, 32 groups):
  h   = GroupNorm(x)                          (512, 1024)
  qkv = Wqkv h + b   (q,k pre-scaled by 64^-0.25 on host, folded into W,b)
  per head:  S^T(s,t) = k^T q                 (no max-subtraction: |S|<~8)
             P^T = exp(S^T)
             a   = v P^T  (with an extra all-ones column in v^T giving Z row)
             a  /= Z  (broadcast via selector matmul)
  out = Wproj a + b_proj;  y = x + out

Matmuls run as float32r (full-rate fp32 on the PE at N>=256); tiny GroupNorm
statistic matmuls run as exact fp32.
"""

import sys
import numpy as np

sys.path.insert(0, "/opt/trn_rl_repo")

import concourse.bacc as bacc
import concourse.bass as bass
import concourse.mybir as mybir
import concourse.tile as tile
from concourse import bass_utils

F32 = mybir.dt.float32
F32R = mybir.dt.float32r
AF = mybir.ActivationFunctionType
ALU = mybir.AluOpType

B, C, HH, WW = 8, 512, 32, 32
T = HH * WW            # 1024
NH = 8                 # heads
CH = C // NH           # 64 per-head dim
NCT = C // 128         # 4 channel tiles
NTT = T // 128         # 8 seq tiles
SCALE = 1.0 / np.sqrt(np.sqrt(CH))
EPS = 1e-5

_CACHE = {}


def build_kernel(debug=False):
    nc = bacc.Bacc(
        "TRN2", target_bir_lowering=False, debug=debug, num_devices=8
    )

    x_d = nc.dram_tensor("x", (C, T), F32, kind="ExternalInput")
    wqkvT_d = nc.dram_tensor("wqkvT", (C, 3 * C + 8), F32R, kind="ExternalInput")
    wprojT_d = nc.dram_tensor("wprojT", (C, C), F32R, kind="ExternalInput")
    # cblob: [gamma | beta | gred] per-128-partition constants
    cblob_d = nc.dram_tensor("cblob", (128, 2 * NCT + 8), F32, kind="ExternalInput")
    # rblob: [bqkv (3C) | bproj (C) | ones (512)] single-partition rows
    rblob_d = nc.dram_tensor("rblob", (1, 3 * C + 8 + C + 512), F32R, kind="ExternalInput")
    gbcast_d = nc.dram_tensor("gbcast", (8, 128), F32, kind="ExternalInput")
    e8_d = nc.dram_tensor("e8", (8, NH * CH), F32R, kind="ExternalInput")
    y_d = nc.dram_tensor("y", (C, T), F32, kind="ExternalOutput")

    with tile.TileContext(nc) as tc:
        with (
            tc.tile_pool(name="single", bufs=1) as single,
            tc.tile_pool(name="xp", bufs=NCT) as xp,
            tc.tile_pool(name="wq", bufs=NCT) as wqp,
            tc.tile_pool(name="wp", bufs=NCT) as wpp,
            tc.tile_pool(name="big", bufs=NCT) as bigp,      # h tiles then a_all
            tc.tile_pool(name="vt", bufs=NTT) as vtp,
            tc.tile_pool(name="qk", bufs=2) as qkp,
            tc.tile_pool(name="pt", bufs=3) as ptp,
            tc.tile_pool(name="aun", bufs=1) as aunp,
            tc.tile_pool(name="zp", bufs=1) as zp,
            tc.tile_pool(name="tmp", bufs=2) as tmpp,
            tc.tile_pool(name="gn", bufs=8) as gnp,
            tc.tile_pool(name="pA", bufs=2, space="PSUM") as ppA,
            tc.tile_pool(name="pB", bufs=2, space="PSUM") as ppB,
        ):
            # ---------------- constant / weight loads ----------------
            cblob = single.tile([128, 2 * NCT + 8], F32, tag="cblob")
            nc.sync.dma_start(out=cblob[:, :], in_=cblob_d.ap())
            gamma = cblob[:, 0:NCT]
            beta = cblob[:, NCT:2 * NCT]
            gred = cblob[:, 2 * NCT:2 * NCT + 8]
            gbcast = single.tile([8, 128], F32, tag="gbcast")
            nc.sync.dma_start(out=gbcast[:, :], in_=gbcast_d.ap())
            e8 = single.tile([8, NH * CH], F32R, tag="e8")
            nc.sync.dma_start(out=e8[:, :], in_=e8_d.ap())
            NQKV = 3 * C + 8
            rblob = single.tile([1, NQKV + C + 512], F32R, tag="rblob")
            nc.sync.dma_start(out=rblob[:, :], in_=rblob_d.ap())
            bqkv = rblob[:, 0:NQKV]
            bproj = rblob[:, NQKV:NQKV + C]
            ones = rblob[:, NQKV + C:NQKV + C + 512]

            wqbig = single.tile([128, NCT, 3 * C + 8], F32R, tag="wqbig")
            nc.sync.dma_start(
                out=wqbig[:, :, :],
                in_=wqkvT_d.ap().rearrange("(c p) t -> p c t", p=128),
            )
            wq_t = [wqbig[:, ct, :] for ct in range(NCT)]
            wpbig = single.tile([128, NCT, C], F32R, tag="wpbig")
            nc.sync.dma_start(
                out=wpbig[:, :, :],
                in_=wprojT_d.ap().rearrange("(c p) t -> p c t", p=128),
            )
            wp_t = [wpbig[:, ct, :] for ct in range(NCT)]
            xbig = single.tile([128, NCT, T], F32, tag="xbig")
            nc.sync.dma_start(
                out=xbig[:, :, :],
                in_=x_d.ap().rearrange("(c p) t -> p c t", p=128),
            )
            x_t = [xbig[:, ct, :] for ct in range(NCT)]

            # ---------------- GroupNorm ----------------
            # per-channel stats via bn_stats/bn_aggr, cross-channel (16/group)
            # reduction + broadcast via tiny exact-fp32 matmuls.
            cs = gnp.tile([128, 2 * NCT], F32, tag="cs")  # cols ct: mean, NCT+ct: E[x^2]
            for ct in range(NCT):
                xr = x_t[ct][:, :].rearrange("p (n f) -> p n f", f=512)
                st = gnp.tile([128, 2, 6], F32, tag="st")
                for sg in range(2):
                    nc.vector.bn_stats(out=st[:, sg, :], in_=xr[:, sg, :])
                mv = gnp.tile([128, 2], F32, tag="mv")
                nc.vector.bn_aggr(out=mv[:, :], in_=st[:, :, :])
                nc.vector.tensor_copy(out=cs[:, ct:ct + 1], in_=mv[:, 0:1])
                nc.vector.tensor_mul(
                    out=cs[:, NCT + ct:NCT + ct + 1], in0=mv[:, 0:1], in1=mv[:, 0:1]
                )
                nc.vector.tensor_add(
                    out=cs[:, NCT + ct:NCT + ct + 1],
                    in0=cs[:, NCT + ct:NCT + ct + 1],
                    in1=mv[:, 1:2],
                )
            gsp = ppA.tile([8, 2 * NCT], F32, tag="A")
            nc.tensor.matmul(gsp[:, :], gred[:, :], cs[:, :], start=True, stop=True)
            gs = gnp.tile([8, 2 * NCT], F32, tag="gs")
            nc.vector.tensor_copy(out=gs[:, :], in_=gsp[:, :])
            # rstd = 1/sqrt(var+eps) with one Newton refinement
            t1 = gnp.tile([8, NCT], F32, tag="t1")
            veps = gnp.tile([8, NCT], F32, tag="veps")
            nc.vector.tensor_mul(out=t1[:, :], in0=gs[:, 0:NCT], in1=gs[:, 0:NCT])
            nc.vector.tensor_sub(out=veps[:, :], in0=gs[:, NCT:], in1=t1[:, :])
            nc.vector.tensor_scalar_add(out=veps[:, :], in0=veps[:, :], scalar1=EPS)
            sq = gnp.tile([8, NCT], F32, tag="sq")
            nc.scalar.activation(out=sq[:, :], in_=veps[:, :], func=AF.Sqrt)
            r0 = gnp.tile([8, NCT], F32, tag="r0")
            nc.vector.reciprocal(out=r0[:, :], in_=sq[:, :])
            nc.vector.tensor_mul(out=t1[:, :], in0=r0[:, :], in1=r0[:, :])
            nc.vector.tensor_mul(out=t1[:, :], in0=t1[:, :], in1=veps[:, :])
            nc.vector.tensor_scalar(
                out=t1[:, :], in0=t1[:, :], scalar1=-0.5, scalar2=1.5,
                op0=ALU.mult, op1=ALU.add,
            )
            nc.vector.tensor_mul(out=r0[:, :], in0=r0[:, :], in1=t1[:, :])
            mr = gnp.tile([8, 2 * NCT], F32, tag="mr")  # cols 2ct: mean_g, 2ct+1: rstd_g
            for ct in range(NCT):
                nc.vector.tensor_copy(
                    out=mr[:, 2 * ct:2 * ct + 1], in_=gs[:, ct:ct + 1]
                )
                nc.vector.tensor_copy(
                    out=mr[:, 2 * ct + 1:2 * ct + 2], in_=r0[:, ct:ct + 1]
                )
            h_t = []
            for ct in range(NCT):
                mrc = ppB.tile([128, 2], F32, tag="B")
                nc.tensor.matmul(
                    mrc[:, :], gbcast[:, :], mr[:, 2 * ct:2 * ct + 2],
                    start=True, stop=True,
                )
                sc = gnp.tile([128, 1], F32, tag="sc")
                sh = gnp.tile([128, 1], F32, tag="sh")
                nc.vector.tensor_mul(
                    out=sc[:, :], in0=mrc[:, 1:2], in1=gamma[:, ct:ct + 1]
                )
                nc.vector.tensor_mul(out=sh[:, :], in0=mrc[:, 0:1], in1=sc[:, :])
                nc.vector.tensor_sub(
                    out=sh[:, :], in0=beta[:, ct:ct + 1], in1=sh[:, :]
                )
                ht = bigp.tile([128, T], F32R, tag="big")
                nc.vector.tensor_scalar(
                    out=ht[:, :], in0=x_t[ct][:, :], scalar1=sc[:, :],
                    scalar2=sh[:, :], op0=ALU.mult, op1=ALU.add,
                )
                h_t.append(ht)

            # ---------------- v^T (+ ones column) ----------------
            # v^T(t, c_v) for all heads at once; ones column at per-head col 64
            # makes the av matmul also produce the softmax denominator Z.
            # v^T for all heads; the v-section of wqkvT carries one extra
            # zero-weight column per head with bias 1.0, so column 64 of each
            # head block is all-ones -> the av matmul also produces Z.
            VW = NH * (CH + 1)  # 520
            vt_t = []
            for tt in range(NTT):
                vps = ppB.tile([128, VW], F32, tag="B")
                for seg in ((0, 512), (512, VW)):
                    dst = vps[:, seg[0]:seg[1]]
                    for ct in range(NCT):
                        nc.tensor.matmul(
                            dst,
                            h_t[ct][:, tt * 128:(tt + 1) * 128],
                            wq_t[ct][:, 2 * C + seg[0]:2 * C + seg[1]],
                            start=(ct == 0), stop=False,
                        )
                    nc.tensor.matmul(
                        dst, ones[0:1, 0:128],
                        bqkv[0:1, 2 * C + seg[0]:2 * C + seg[1]],
                        start=False, stop=True,
                    )
                vt = vtp.tile([128, VW], F32R, tag="vt")
                nc.vector.tensor_copy(out=vt[:, :], in_=vps[:, :])
                vt_t.append(vt)

            # ---------------- per-pair qkv + attention ----------------
            # One big a_un tile: head h occupies [:, h, :]; row 64 holds the
            # softmax denominator Z so a single DMA can gather all 8 Z rows.
            aunbig = aunp.tile([CH + 1, NH, T], F32, tag="aun")
            zall = zp.tile([8, T], F32, tag="z")
            for p in range(NH // 2):
                # q and k for heads 2p, 2p+1: psum rows 0..63 = even head,
                # 64..127 = odd head -> scores matmuls stay partition-aligned.
                qps = ppA.tile([128, T], F32, tag="A")
                kps = ppB.tile([128, T], F32, tag="B")
                for psum, off in ((qps, 256 * p), (kps, 256 * p + 128)):
                    for nh2 in range(2):
                        dst = psum[:, nh2 * 512:(nh2 + 1) * 512]
                        for ct in range(NCT):
                            nc.tensor.matmul(
                                dst,
                                wq_t[ct][:, off:off + 128],
                                h_t[ct][:, nh2 * 512:(nh2 + 1) * 512],
                                start=(ct == 0), stop=False,
                            )
                        nc.tensor.matmul(
                            dst,
                            bqkv[0:1, off:off + 128],
                            ones[0:1, 0:512],
                            start=False, stop=True,
                        )
                qp_s = qkp.tile([128, T], F32R, tag="qpair")
                nc.vector.tensor_copy(out=qp_s[:, :], in_=qps[:, :])
                kp_s = qkp.tile([128, T], F32R, tag="kpair")
                nc.vector.tensor_copy(out=kp_s[:, :], in_=kps[:, :])

                for hl in range(2):
                    h_ = 2 * p + hl
                    base = 64 * hl
                    avps = ppB.tile([CH + 1, T], F32, tag="B")
                    for st_ in range(NTT):
                        scps = ppA.tile([128, T], F32, tag="A")
                        for nh2 in range(2):
                            nc.tensor.matmul(
                                scps[:, nh2 * 512:(nh2 + 1) * 512],
                                kp_s[base:base + 64, st_ * 128:(st_ + 1) * 128],
                                qp_s[base:base + 64, nh2 * 512:(nh2 + 1) * 512],
                                start=True, stop=True,
                            )
                        pt = ptp.tile([128, T], F32R, tag="pt")
                        nc.scalar.activation(out=pt[:, :], in_=scps[:, :], func=AF.Exp)
                        for nh2 in range(2):
                            nc.tensor.matmul(
                                avps[:, nh2 * 512:(nh2 + 1) * 512],
                                vt_t[st_][:, h_ * (CH + 1):(h_ + 1) * (CH + 1)],
                                pt[:, nh2 * 512:(nh2 + 1) * 512],
                                start=(st_ == 0), stop=(st_ == NTT - 1),
                            )
                    nc.vector.tensor_copy(
                        out=aunbig[:, h_, :], in_=avps[:, :]
                    )

            # ---------------- softmax normalization ----------------
            nc.sync.dma_start(out=zall[:, :], in_=aunbig[CH:CH + 1, :, :])
            invz = zp.tile([8, T], F32R, tag="invz")
            with nc.allow_low_precision(reason="fp32r matmul operand"):
                nc.vector.reciprocal(out=invz[:, :], in_=zall[:, :])
            a_all = []
            for ct in range(NCT):
                a_all.append(bigp.tile([128, T], F32R, tag="big", name=f"aall{ct}"))
            for h_ in range(NH):
                zb = ppA.tile([CH, T], F32, tag="A")
                for nh2 in range(2):
                    nc.tensor.matmul(
                        zb[:, nh2 * 512:(nh2 + 1) * 512],
                        e8[:, h_ * CH:(h_ + 1) * CH],
                        invz[:, nh2 * 512:(nh2 + 1) * 512],
                        start=True, stop=True,
                    )
                if h_ % 2 == 0:
                    dst = a_all[h_ // 2][0:CH, :]
                    nc.vector.tensor_mul(
                        out=dst, in0=aunbig[0:CH, h_, :], in1=zb[:, :]
                    )
                else:
                    atmp = tmpp.tile([CH, T], F32R, tag="atmp")
                    nc.vector.tensor_mul(
                        out=atmp[:, :], in0=aunbig[0:CH, h_, :], in1=zb[:, :]
                    )
                    nc.sync.dma_start(
                        out=a_all[h_ // 2][CH:2 * CH, :], in_=atmp[:, :]
                    )

            # ---------------- out projection + residual ----------------
            for m in range(NCT):
                pps = ppA.tile([128, T], F32, tag="A")
                for nh2 in range(2):
                    dst = pps[:, nh2 * 512:(nh2 + 1) * 512]
                    for ck in range(NCT):
                        nc.tensor.matmul(
                            dst,
                            wp_t[ck][:, m * 128:(m + 1) * 128],
                            a_all[ck][:, nh2 * 512:(nh2 + 1) * 512],
                            start=(ck == 0), stop=False,
                        )
                    nc.tensor.matmul(
                        dst,
                        bproj[0:1, m * 128:(m + 1) * 128],
                        ones[0:1, 0:512],
                        start=False, stop=True,
                    )
                nc.vector.tensor_add(
                    out=x_t[m][:, :], in0=pps[:, :], in1=x_t[m][:, :]
                )
                nc.sync.dma_start(
                    out=y_d.ap()[m * 128:(m + 1) * 128, :], in_=x_t[m][:, :]
                )

    nc.compile()
    return nc


def make_in_maps(x, gn_weight, gn_bias, w_qkv, b_qkv, w_proj, b_proj):
    x = np.asarray(x, dtype=np.float32)
    w_qkv = np.asarray(w_qkv, dtype=np.float32)
    b_qkv = np.asarray(b_qkv, dtype=np.float32)
    scale = np.float32(SCALE)
    wq = w_qkv.copy()
    bq = b_qkv.copy()
    for hd in range(NH):
        sl = slice(3 * CH * hd, 3 * CH * hd + 2 * CH)  # q,k rows of this head
        wq[sl] *= scale
        bq[sl] *= scale
    # Column order expected by the kernel: per head-pair p the contiguous
    # blocks [q(2p) | q(2p+1) | k(2p) | k(2p+1)] (256 cols each), then all
    # v blocks. Makes every PE stationary-operand slice a single free dim.
    perm = []
    for p in range(NH // 2):
        for hd in (2 * p, 2 * p + 1):
            perm.extend(range(3 * CH * hd, 3 * CH * hd + CH))          # q
        for hd in (2 * p, 2 * p + 1):
            perm.extend(range(3 * CH * hd + CH, 3 * CH * hd + 2 * CH))  # k
    for hd in range(NH):
        perm.extend(range(3 * CH * hd + 2 * CH, 3 * CH * hd + 3 * CH))  # v
    perm = np.asarray(perm)
    wq = wq[perm]
    bq = bq[perm]
    wprojT = np.ascontiguousarray(np.asarray(w_proj, np.float32).T)  # (C, C)
    gamma = np.asarray(gn_weight, np.float32).reshape(NCT, 128).T
    beta = np.asarray(gn_bias, np.float32).reshape(NCT, 128).T
    gred = np.zeros((128, 8), np.float32)
    gbcast = np.zeros((8, 128), np.float32)
    for c in range(128):
        gred[c, c // 16] = 1.0 / 16.0
        gbcast[c // 16, c] = 1.0
    e8 = np.zeros((8, NH * CH), np.float32)
    for g in range(8):
        e8[g, g * CH:(g + 1) * CH] = 1.0
    cblob = np.ascontiguousarray(
        np.concatenate([gamma, beta, gred], axis=1)
    )                                                        # (128, 16)
    # v-section gains a zero-weight column with bias 1.0 per head (the Z
    # column of v^T); qk section stays 1024 wide.
    wq2 = np.zeros((C, 3 * C + 8), np.float32)
    bq2 = np.zeros(3 * C + 8, np.float32)
    wq2[:, 0:2 * C] = wq.T[:, 0:2 * C]
    bq2[0:2 * C] = bq[0:2 * C]
    for hd in range(NH):
        wq2[:, 2 * C + 65 * hd:2 * C + 65 * hd + CH] = \
            wq.T[:, 2 * C + CH * hd:2 * C + CH * (hd + 1)]
        bq2[2 * C + 65 * hd:2 * C + 65 * hd + CH] = \
            bq[2 * C + CH * hd:2 * C + CH * (hd + 1)]
        bq2[2 * C + 65 * hd + CH] = 1.0
    wqkvT2 = np.ascontiguousarray(wq2)
    rblob = np.concatenate(
        [bq2, np.asarray(b_proj, np.float32), np.ones(512, np.float32)]
    ).reshape(1, -1)

    common = dict(
        wqkvT=wqkvT2, wprojT=wprojT, cblob=cblob, rblob=rblob,
        gbcast=gbcast, e8=e8,
    )
    in_maps = []
    for b in range(B):
        m = dict(common)
        m["x"] = np.ascontiguousarray(x[b].reshape(C, T))
        in_maps.append(m)
    return in_maps


def kernel(x, gn_weight, gn_bias, w_qkv, b_qkv, w_proj, b_proj, _trace=False):
    if "nc" not in _CACHE:
        _CACHE["nc"] = build_kernel()
    nc = _CACHE["nc"]
    in_maps = make_in_maps(x, gn_weight, gn_bias, w_qkv, b_qkv, w_proj, b_proj)
    res = bass_utils.run_bass_kernel_spmd(
        nc, in_maps, core_ids=list(range(B)), trace=_trace
    )
    out = np.stack([r["y"].reshape(C, HH, WW) for r in res.results], axis=0)
    if _trace:
        _CACHE["last_result"] = res
    return out
